# revision 81
# baseline (speedup 1.0000x reference)
# 2D DCT-II [4096,4096] on 8 NeuronCores — v5.2 "hybrid, transpose-free" (bf16).
#
# Column space (storage positions) is split: the first NCHN*512 columns are
# REPLICATED to every core, which computes a narrow stage-1 DFT64 (only its
# own 8 wire slots, q-group-packed so PSUM accumulation fills all 128 output
# partitions) fed by the DMA channel; the remaining columns go through the
# baseline pencil path (own-share full S1 -> AllToAll) on the collective
# channel. Downstream both corner turns are folded into flipped matmuls
# (data as lhsT), so there are no PE transposes: S2f emits x3 with storage
# columns on partitions; ColS1f (wc1 columns permuted to i = 4*pp+2*par+bit)
# emits o3d in DRAM so each ColS2 rhs tile is one contiguous read. ColS2
# merges Hermitian pairs (c, 64-c) sharing one rhs. Everything is
# column-chunk pipelined; PE instruction count is kept low because the PE
# sequencer, not the PE array, is the scarce resource.
import numpy as np
import ml_dtypes
import concourse.bacc as bacc
import concourse.tile as tile
import concourse.mybir as mybir
from concourse import bass_utils

M = N = 4096
NC = 8
NCH = 8           # total 512-column chunks
CH = N // NCH     # 512
NCHN = 4          # narrow (replicated) chunks; rest go via AllToAll
NA = NCH - NCHN   # A2A chunks
OWN = NA * CH // NC   # own-share columns per core on the A2A path

BF = ml_dtypes.bfloat16

# ---------------- weight/permutation construction (host) ----------------
W64C = np.exp(-2j * np.pi * np.arange(64)[:, None] * np.arange(64)[None, :] / 64)


def makhoul_perm(n):
    p = np.empty(n, dtype=np.int64)
    half = n // 2
    p[:half] = 2 * np.arange(half)
    p[half:] = 2 * (n - 1 - np.arange(half, n)) + 1
    return p


ROWP = makhoul_perm(M)
COLP = makhoul_perm(N)
MQ = (np.arange(N) % 64) * 64 + np.arange(N) // 64   # storage pos of v-col m


def wire_slots(r):
    if r == 0:
        return [(0, 0), (32, 0), (1, 0), (1, 1), (2, 0), (2, 1), (3, 0), (3, 1)]
    return [(4 * r + j // 2, j % 2) for j in range(8)]


def out_chat(r, o):
    j, v = o // 2, o % 2
    if r == 0 and j == 0:
        return 0 if v == 0 else 32
    ct = 4 * r + j
    return ct if v == 0 else 64 - ct


def w1_matrix():
    """lhsT for the A2A-path S1: [128, 128] block-diag; cols = wire planes."""
    w = np.zeros((64, 64))
    for r in range(NC):
        for ell, (ct, im) in enumerate(wire_slots(r)):
            col = 8 * r + ell
            w[:, col] = W64C[:, ct].imag if im else W64C[:, ct].real
    full = np.zeros((128, 128))
    full[:64, :64] = w
    full[64:, 64:] = w
    return full


def w1n_matrix(r):
    """[128 in, 8 jq, 128 out] narrow-S1 weights for core r.

    in partition = a + 64*par (matrix-row m = 64a + b, b = 2q + par,
    q = 8*qg + jq); out col = (2*jq + par) + 16*lhat."""
    w = np.zeros((8, 128, 128))
    for jq in range(8):
        for lhat, (ct, im) in enumerate(wire_slots(r)):
            coeff = W64C[:, ct].imag if im else W64C[:, ct].real
            for par in range(2):
                w[jq, 64 * par:64 * par + 64, (2 * jq + par) + 16 * lhat] = coeff
    return np.ascontiguousarray(w.transpose(1, 0, 2))


def wr2_weights(r):
    """[8, 128, 64] for core r."""
    b = np.arange(64)
    d = np.arange(64)
    out = np.zeros((8, 128, 64))
    for o in range(8):
        chat = out_chat(r, o)
        alpha = np.exp(-1j * np.pi * chat / (2 * M))
        beta = np.exp(-1j * np.pi * d / 128)
        g = (alpha * np.exp(-2j * np.pi * b[:, None] * chat / M)
             * W64C[b][:, d] * beta[None, :])
        j, v = o // 2, o % 2
        if r == 0 and j == 0:
            if v == 0:
                out[o, :64] = g.real
            else:
                out[o, 64:] = g.real
        else:
            out[o, :64] = g.real
            out[o, 64:] = -g.imag if v == 0 else g.imag
    return out


def wr2m_matrix(r):
    """[128 in, 4 op, 128 out]: merged pairs concat(wr2[2op], wr2[2op+1])."""
    wr2 = wr2_weights(r)
    out = np.zeros((128, 4, 128))
    for op in range(4):
        out[:, op, :64] = wr2[2 * op]
        out[:, op, 64:] = wr2[2 * op + 1]
    return np.ascontiguousarray(out)


def wc1_matrix():
    w = np.zeros((64, 64))
    for ct in range(32):
        w[:, 2 * ct] = W64C[:, ct].real
        if ct == 0:
            w[:, 1] = W64C[:, 32].real
        else:
            w[:, 2 * ct + 1] = W64C[:, ct].imag
    full = np.zeros((128, 128))
    full[:64, :64] = w
    full[64:, 64:] = w
    return full


def wc1f_matrix():
    """wc1 with columns permuted: i = 4*pp + 2*par + bit (pos = 2pp + par)."""
    wc1 = wc1_matrix()
    out = np.zeros((128, 128))
    for pp in range(32):
        for par in range(2):
            for bit in range(2):
                out[:, 4 * pp + 2 * par + bit] = wc1[:, (2 * pp + par) + 64 * bit]
    return out


def wc2_weights():
    """[64, 128, 64], uniform across cores; bt_of_p row permutation baked in."""
    b = np.arange(64)
    d = np.arange(64)
    p = np.arange(64)
    bt = 2 * (p % 32) + p // 32
    out = np.zeros((64, 128, 64))
    for oc in range(64):
        conj = oc > 32
        alpha = np.exp(-1j * np.pi * oc / (2 * N))
        beta = np.exp(-1j * np.pi * d / 128)
        g = (alpha * np.exp(-2j * np.pi * b[:, None] * oc / N)
             * W64C[b][:, d] * beta[None, :])
        if oc == 0:
            out[oc, :64] = g.real[bt]
        elif oc == 32:
            out[oc, 64:] = g.real[bt]
        else:
            out[oc, :64] = g.real[bt]
            out[oc, 64:] = (g.imag if conj else -g.imag)[bt]
    return out


def wc2m_matrix():
    """[128 in, 32 chat, 128 out]: pairs (chat, 64-chat) share rhs x4[chat]."""
    wc2 = wc2_weights()
    out = np.zeros((128, 32, 128))
    for chat in range(32):
        oc2 = 64 - chat if chat > 0 else 32
        out[:, chat, :64] = wc2[chat]
        out[:, chat, 64:] = wc2[oc2]
    return np.ascontiguousarray(out)


# ---------------- bass kernel ----------------
_BUILT = {}


def build_nc():
    dt = mybir.dt
    bf = dt.bfloat16
    nc = bacc.Bacc("TRN2", target_bir_lowering=False, debug=False, num_devices=NC)

    xin = nc.dram_tensor("xin", [128, 32, NCHN * CH], bf, kind="ExternalInput")
    xina = nc.dram_tensor("xina", [128, 32, OWN], bf, kind="ExternalInput")
    w1 = nc.dram_tensor("w1", [128, 128], bf, kind="ExternalInput")
    w1n = nc.dram_tensor("w1n", [128, 8, 128], bf, kind="ExternalInput")
    wr2m = nc.dram_tensor("wr2m", [128, 4, 128], bf, kind="ExternalInput")
    wc1f = nc.dram_tensor("wc1f", [128, 128], bf, kind="ExternalInput")
    wc2m = nc.dram_tensor("wc2m", [128, 32, 128], bf, kind="ExternalInput")
    y = nc.dram_tensor("y", [128, 32, 512], bf, kind="ExternalOutput")

    with tile.TileContext(nc) as tc:
        cp_eng = [nc.vector.tensor_copy, nc.scalar.copy]

        def copy(i, dst, src):
            cp_eng[i % 2](dst, src)

        with tc.tile_pool(name="dram", bufs=1, space="DRAM") as dram:
            z1 = dram.tile([64, 64, OWN], bf)
            z2 = dram.tile([64, 64, OWN], bf)
            z1f = dram.tile([8, 4, 16, NCHN * CH], bf)   # [lhat, qg, bhat, c]
            o3d = dram.tile([128, 32, 512], bf)          # [i, cs, rows]
            with tc.tile_pool(name="wts", bufs=1) as wts:
                w1s = wts.tile([128, 128], bf, tag="w1s")
                w1ns = wts.tile([128, 8, 128], bf, tag="w1ns")
                wr2ms = wts.tile([128, 4, 128], bf, tag="wr2ms")
                wc1fs = wts.tile([128, 128], bf, tag="wc1fs")
                wc2ms = wts.tile([128, 32, 128], bf, tag="wc2ms")
                nc.scalar.dma_start(out=w1s[:], in_=w1[:])
                nc.scalar.dma_start(out=wr2ms[:], in_=wr2m[:])
                nc.scalar.dma_start(out=w1ns[:], in_=w1n[:])
                nc.scalar.dma_start(out=wc1fs[:], in_=wc1f[:])

                with (
                    tc.tile_pool(name="persist", bufs=1) as pp,
                    tc.tile_pool(name="xsring", bufs=2) as xsring,
                    tc.tile_pool(name="ring", bufs=2) as ring,
                    tc.tile_pool(name="pa2a", bufs=2) as pa2a,
                    tc.tile_pool(name="pA", bufs=4) as pA,
                    tc.tile_pool(name="pAo", bufs=2) as pAo,
                    tc.tile_pool(name="stg", bufs=2) as stg,
                ):
                    x3 = pp.tile([128, 32, 512], bf, tag="x3")

                    _pscm = [tc.tile_pool(name=n, space="PSUM", bufs=b)
                             for n, b in [("psS1", 1), ("psX", 2), ("psC", 2)]]
                    psS1, psX, psC = [c.__enter__() for c in _pscm]

                    # ---------- A2A path: own-share S1 + z1 + collective -----
                    zv = z1[:].rearrange("pl (q t) c -> pl q t c", t=2)
                    for g in range(4):
                        xsa = pA.tile([128, 8, OWN], bf, tag="xsa")
                        nc.sync.dma_start(out=xsa[:],
                                          in_=xina[:, 8 * g:8 * (g + 1), :])
                        o1a = pAo.tile([128, 8, OWN], bf, tag="o1a")
                        for qpl in range(4):
                            ps = psC.tile([128, 2, OWN], dt.float32, tag="ps3")
                            nc.tensor.matmul(ps[:], w1s[:],
                                             xsa[:, 2 * qpl:2 * qpl + 2, :],
                                             start=True, stop=True)
                            copy(qpl, o1a[:, 2 * qpl:2 * qpl + 2, :], ps[:])
                        qs = slice(8 * g, 8 * (g + 1))
                        nc.scalar.dma_start(out=zv[:, qs, 0, :],
                                            in_=o1a[:64, :, :])
                        nc.scalar.dma_start(out=zv[:, qs, 1, :],
                                            in_=o1a[64:, :, :])

                    nc.gpsimd.collective_compute(
                        "AllToAll", mybir.AluOpType.bypass,
                        replica_groups=[list(range(NC))],
                        ins=[z1[:].opt()], outs=[z2[:].opt()])
                    z2v = z2[:].rearrange("(r l) b c -> r l b c", r=8)
                    nc.scalar.dma_start(out=wc2ms[:], in_=wc2m[:])

                    # ---------- shared stage bodies ----------
                    def s2f(ch, x2):
                        """x2 [128 (b,v), 4 j, 512 c] -> x3[:, 4ch..4ch+4, :]"""
                        for cb in range(4):
                            cs = 4 * ch + cb
                            ps = psX.tile([128, 512], dt.float32, tag="ps2")
                            for j in range(4):
                                nc.tensor.matmul(
                                    ps[:, 128 * j:128 * (j + 1)],
                                    x2[:, j, 128 * cb:128 * (cb + 1)],
                                    wr2ms[:, j, :], start=True, stop=True)
                            copy(cb, x3[:, cs, :], ps[:])

                    def p7f(ch):
                        """x3 cs-slice -> o3d[:, 4ch..4ch+4, :] via wc1f."""
                        st = stg.tile([128, 4, 512], bf, tag="stg")
                        for cb in range(4):
                            cs = 4 * ch + cb
                            ps = psC.tile([128, 512], dt.float32, tag="ps3")
                            nc.tensor.matmul(ps[:], wc1fs[:], x3[:, cs, :],
                                             start=True, stop=True)
                            copy(cb, st[:, cb, :], ps[:])
                        nc.scalar.dma_start(
                            out=o3d[:, 4 * ch:4 * (ch + 1), :], in_=st[:])

                    # ---------- narrow chunks (DMA channel) ----------
                    for ch in range(NCHN):
                        cs0 = slice(CH * ch, CH * (ch + 1))
                        xs = xsring.tile([128, 32, CH], bf, tag="xs")
                        for h_ in range(2):
                            nc.sync.dma_start(
                                out=xs[:, 16 * h_:16 * (h_ + 1), :],
                                in_=xin[:, 16 * h_:16 * (h_ + 1), cs0])
                        ps = psS1.tile([128, 4, CH], dt.float32, tag="ps1")
                        for jq in range(8):
                            for qg in range(4):
                                nc.tensor.matmul(ps[:, qg, :], w1ns[:, jq, :],
                                                 xs[:, 8 * qg + jq, :],
                                                 start=(jq == 0),
                                                 stop=(jq == 7))
                        o1 = ring.tile([128, 4, CH], bf, tag="o1")
                        copy(0, o1[:, 0:2, :], ps[:, 0:2, :])
                        copy(1, o1[:, 2:4, :], ps[:, 2:4, :])
                        for l in range(8):
                            nc.sync.dma_start(
                                out=z1f[l, :, :, cs0].rearrange(
                                    "qg bh c -> bh qg c"),
                                in_=o1[16 * l:16 * l + 16, :, :])
                        x2 = ring.tile([128, 4, CH], bf, tag="x2")
                        zsrc = z1f[:, :, :, cs0].rearrange(
                            "(j v) qg bh c -> (v qg bh) j c", v=2)
                        nc.sync.dma_start(out=x2[:], in_=zsrc)
                        s2f(ch, x2)
                        p7f(ch)

                    # ---------- A2A chunks: prefetch + sweeps ----------
                    x2a_tiles = []
                    for k in range(NA):
                        x2a = pa2a.tile([128, 4, CH], bf, tag="x2a")
                        for si in range(CH // OWN):
                            sc = (CH // OWN) * k + si
                            for half in range(2):
                                src = z2v[sc, half::2].rearrange("l b c -> b l c")
                                nc.sync.dma_start(
                                    out=x2a[64 * half:64 * (half + 1), :,
                                            si * OWN:(si + 1) * OWN], in_=src)
                        x2a_tiles.append(x2a)
                    for k in range(NA):
                        s2f(NCHN + k, x2a_tiles[k])
                        p7f(NCHN + k)

                    for c in reversed(_pscm):
                        c.__exit__(None, None, None)

                    # ---- tail: x4 contiguous reads + ColS2 (merged pairs) ----
                    cp3 = [nc.vector.tensor_copy, nc.scalar.copy,
                           nc.vector.tensor_copy]
                    with (
                        tc.tile_pool(name="tx4", bufs=8) as tx4,
                        tc.tile_pool(name="tyb", bufs=4) as tyb,
                        tc.tile_pool(name="psD", bufs=3, space="PSUM") as psD,
                    ):
                        x4s = {}
                        yb = None
                        for jp in range(23):
                            if jp < 16:
                                x4 = tx4.tile([128, 2, 512], bf, tag="x4")
                                nc.sync.dma_start(
                                    out=x4[:],
                                    in_=o3d[8 * jp:8 * (jp + 1)].rearrange(
                                        "(j pb) cs r -> (pb cs) j r", j=2))
                                x4s[jp] = x4
                            if jp >= 7:
                                jq = jp - 7
                                x4 = x4s.pop(jq)
                                if jq % 2 == 0:
                                    yb = tyb.tile([128, 4, 512], bf, tag="yb")
                                psy = psD.tile([128, 2, 512], dt.float32,
                                               tag="ps4")
                                for t in range(2):
                                    chat = 2 * jq + t
                                    nc.tensor.matmul(psy[:, t, :],
                                                     wc2ms[:, chat, :],
                                                     x4[:, t, :],
                                                     start=True, stop=True)
                                h = jq % 2
                                cp3[jq % 3](yb[:, 2 * h:2 * h + 2, :], psy[:])
                                if h == 1:
                                    nc.sync.dma_start(
                                        out=y[:, 2 * (jq - 1):2 * (jq + 1), :],
                                        in_=yb[:])

    nc.compile()
    return nc


# ---------------- host-side prep / unscramble ----------------
def _prep_weights():
    if "wc1f" in _BUILT:
        return
    _BUILT["w1"] = np.ascontiguousarray(w1_matrix().astype(BF))
    _BUILT["wc1f"] = np.ascontiguousarray(wc1f_matrix().astype(BF))
    _BUILT["wc2m"] = np.ascontiguousarray(wc2m_matrix().astype(BF))
    _BUILT["w1n"] = [w1n_matrix(r).astype(BF) for r in range(NC)]
    _BUILT["wr2m"] = [wr2m_matrix(r).astype(BF) for r in range(NC)]


def make_in_maps(x):
    """x float32 [4096, 4096] -> per-core input dicts."""
    _prep_weights()
    x = np.asarray(x, dtype=np.float32)
    vr = x[ROWP, :]
    xs = np.empty_like(vr)
    xs[:, MQ] = vr[:, COLP]
    a3 = xs.reshape(64, 64, N)
    xin_full = np.empty((128, 32, N), dtype=BF)
    xin_full[:64] = a3[:, 0::2, :]
    xin_full[64:] = a3[:, 1::2, :]
    xin = np.ascontiguousarray(xin_full[:, :, :NCHN * CH])
    maps = []
    for c in range(NC):
        c0 = NCHN * CH + c * OWN
        maps.append({
            "xin": xin,
            "xina": np.ascontiguousarray(xin_full[:, :, c0:c0 + OWN]),
            "w1": _BUILT["w1"],
            "w1n": _BUILT["w1n"][c],
            "wr2m": _BUILT["wr2m"][c],
            "wc1f": _BUILT["wc1f"],
            "wc2m": _BUILT["wc2m"],
        })
    return maps


def assemble(results):
    """results: list of per-core dicts with 'y' [128, 32, 512] bf16."""
    yfull = np.empty((M, N), dtype=np.float32)
    p = np.arange(128)
    kps = np.empty((128, 32), dtype=np.int64)
    for chat in range(32):
        oc = np.where(p < 64, chat, 32 if chat == 0 else 64 - chat)
        kps[:, chat] = oc + 64 * (p % 64)
    for r in range(NC):
        ks = np.empty(512, dtype=np.int64)
        for op in range(4):
            for par in range(2):
                o = 2 * op + par
                ks[op * 128 + par * 64:op * 128 + par * 64 + 64] = (
                    out_chat(r, o) + 64 * np.arange(64))
        yr = np.asarray(results[r]["y"]).astype(np.float32)  # [128, 32, 512]
        flat = yr.transpose(2, 0, 1).reshape(512, 4096)      # [row, (p, chat)]
        yfull[np.ix_(ks, kps.reshape(-1))] = flat
    return yfull


def kernel(x, expkM=None, expkN=None, trace=False):
    if "nc" not in _BUILT:
        _BUILT["nc"] = build_nc()
    nc = _BUILT["nc"]
    in_maps = make_in_maps(x)
    res = bass_utils.run_bass_kernel_spmd(nc, in_maps, core_ids=list(range(NC)),
                                          trace=trace)
    _BUILT["last_res"] = res
    return assemble(res.results).astype(np.float32)
